# revision 19
# baseline (speedup 1.0000x reference)
"""Causal linear attention (ELU+1 feature map) on 8 trn2 NeuronCores.

Sharding: core i handles batch b=i//2, sequence half h=i%2 (T=2048 -> 1024
tokens/core).  Second-half cores recompute the first half's running state
S0 = sum_tau phi(k_tau) [v_tau, 1]  (128x129, col 128 = z) from k/v of the
first half; first-half cores get zeroed aux inputs so their S0 == 0.

Math per core (chunk C=128, 8 own chunks + 8 "pre" state-only chunks):
  phi(y) = min(exp(y), 1) + relu(y)            (== ELU(y)+1 exactly)
  A^T_c = K_c Q_c^T ; mask (tau<=t) fused into PSUM->SBUF bf16 copy
  O_c = Amask^T.T @ [V_c, 1] + Q_c @ (Se + So)  (den accumulates in col 128)
  Se/So: parity-split state accumulators (halves the snapshot chain)
  out_c = O_c[:, :128] * (1 / O_c[:, 128])

All inputs in one bf16 pack, DMA'd in need-order pieces across sync(HWDGE)
and Pool(SWDGE); output staged and written in four pieces.
"""

import numpy as np

B, T, D, DV = 4, 2048, 128, 128
H = T // 2          # tokens per core
C = 128             # chunk
NCH = H // C        # chunks per half
NCORES = 8
VW = DV + 1

# bf16 pack columns: [WTb | mask | ident | bias | kTp | qT | kT | vp | v]
OFF_WTB = 0
OFF_MASK = OFF_WTB + D
OFF_ID = OFF_MASK + C
OFF_BIAS = OFF_ID + C
OFF_KTP = OFF_BIAS + 1
OFF_QT = OFF_KTP + H
OFF_KT = OFF_QT + H
OFF_VP = OFF_KT + H
OFF_V = OFF_VP + NCH * VW
B16_COLS = OFF_V + NCH * VW

_cache = {}


def _build():
    import concourse.bacc as bacc
    import concourse.tile as tile
    from concourse import mybir
    from bass_rust import add_dep_helper

    F32 = mybir.dt.float32
    BF16 = mybir.dt.bfloat16
    AF = mybir.ActivationFunctionType
    MUL = mybir.AluOpType.mult

    nc = bacc.Bacc(None, target_bir_lowering=False, debug=False,
                   num_devices=NCORES)

    bin_ = nc.declare_dram_parameter("bin", [D, B16_COLS], BF16, isOutput=False)
    btile = nc.declare_dram_parameter("btile", [1, H], BF16, isOutput=False)
    out = nc.declare_dram_parameter("out", [C, NCH * DV], F32, isOutput=True)

    with tile.TileContext(nc) as tc:
        with (
            tc.tile_pool(name="cst", bufs=1) as cst,
            tc.tile_pool(name="io", bufs=1) as io,
            tc.tile_pool(name="phi", bufs=1) as phip,
            tc.tile_pool(name="am", bufs=NCH) as amp,
            tc.tile_pool(name="wrk", bufs=3) as wrk,
            tc.tile_pool(name="ps_pre", bufs=2, space="PSUM") as ps_pre,
            tc.tile_pool(name="ps_s", bufs=2, space="PSUM") as ps_s,
            tc.tile_pool(name="ps_a", bufs=2, space="PSUM") as ps_a,
            tc.tile_pool(name="ps_o", bufs=2, space="PSUM") as ps_o,
        ):
            # ---- warm the ACT table while DMAs run ----
            s_warm = cst.tile([D, 1], F32)
            nc.vector.memset(s_warm, 0.0)
            s_warm2 = cst.tile([D, 1], BF16)
            nc.scalar.activation(s_warm2, s_warm, AF.Exp)

            # ---- loads: need-ordered pieces across sync + pool ----
            s_b16 = io.tile([D, B16_COLS], BF16)
            s_btile = cst.tile([1, H], BF16)
            s_ones = cst.tile([1, C], BF16)
            nc.sync.dma_start(out=s_btile, in_=btile[:, :])
            nc.vector.memset(s_ones, 1.0)
            nc.sync.dma_start(out=s_b16[:, 0:OFF_QT], in_=bin_[:, 0:OFF_QT])
            nc.gpsimd.dma_start(out=s_b16[:, OFF_QT:OFF_KT],
                                in_=bin_[:, OFF_QT:OFF_KT])
            nc.sync.dma_start(out=s_b16[:, OFF_KT:OFF_VP],
                              in_=bin_[:, OFF_KT:OFF_VP])
            nc.gpsimd.dma_start(out=s_b16[:, OFF_VP:OFF_V],
                                in_=bin_[:, OFF_VP:OFF_V])
            nc.sync.dma_start(out=s_b16[:, OFF_V:B16_COLS],
                              in_=bin_[:, OFF_V:B16_COLS])

            s_bias = s_b16[:, OFF_BIAS:OFF_BIAS + 1]
            sWTb = s_b16[:, OFF_WTB:OFF_WTB + D]
            s_mask = s_b16[:, OFF_MASK:OFF_MASK + C]
            s_ident = s_b16[:, OFF_ID:OFF_ID + C]

            def vsl(c):
                return s_b16[:, OFF_V + VW * c:OFF_V + VW * (c + 1)]

            def vpsl(c):
                return s_b16[:, OFF_VP + VW * c:OFF_VP + VW * (c + 1)]

            # parity-split state accumulators [D, DV+1]
            Se = ps_s.tile([D, DV + 1], F32, tag="s")
            So = ps_s.tile([D, DV + 1], F32, tag="s")
            Sp = [Se, So]
            started = [False, False]
            s_first = [None, None]

            # ---- token-major pre for K_pre (state recompute path first) ----
            phi_t = phip.tile([C, H], BF16)       # K_tok_pre chunks
            e_t = phip.tile([C, H], BF16)
            r_t = phip.tile([C, H], BF16)
            for j in range(H // 512):
                pst = ps_pre.tile([C, 512], F32, tag="pre")
                prev = nc.tensor.matmul(pst, s_ones,
                                 s_btile[:, 512 * j:512 * (j + 1)],
                                 start=True, stop=False)
                for cc in range(4):
                    c = 4 * j + cc
                    mm_c = nc.tensor.matmul(pst[:, C * cc:C * (cc + 1)],
                                     s_b16[:, OFF_KTP + C * c:OFF_KTP + C * (c + 1)],
                                     sWTb, start=False, stop=(cc == 3))
                    add_dep_helper(mm_c.ins, prev.ins, sync=False,
                                   reason="psum group order")
                    prev = mm_c
                sl = slice(512 * j, 512 * (j + 1))
                nc.scalar.activation(e_t[:, sl], pst, AF.Exp)
                nc.vector.tensor_scalar_max(r_t[:, sl], pst, 0.0)
                nc.gpsimd.tensor_scalar_min(e_t[:, sl], e_t[:, sl], 1.0)
                nc.vector.tensor_add(phi_t[:, sl], e_t[:, sl], r_t[:, sl])
                # pre-half state contributions (zeros on half-0 cores)
                for cc in range(4):
                    c = 4 * j + cc
                    p = c % 2
                    mm_s = nc.tensor.matmul(Sp[p], phi_t[:, C * c:C * (c + 1)],
                                     vpsl(c),
                                     start=(not started[p]), stop=False,
                                     skip_group_check=True)
                    if started[p]:
                        add_dep_helper(mm_s.ins, s_first[p].ins, sync=False,
                                       reason="psum group order")
                    s_first[p] = mm_s
                    started[p] = True

            # ---- feature-major phi for own q, k (512-col pipeline) ----
            phi_f = phip.tile([D, 2 * H], BF16)   # [Q^T | K^T]
            e_f = phip.tile([D, 2 * H], BF16)
            r_f = phip.tile([D, 2 * H], BF16)

            def phi_slice(i, off, j):
                pre = ps_pre.tile([D, 512], F32, tag="pre")
                nc.tensor.matmul(pre, sWTb,
                                 s_b16[:, off + 512 * j:off + 512 * (j + 1)],
                                 start=True, stop=True)
                sl = slice(H * i + 512 * j, H * i + 512 * (j + 1))
                nc.scalar.activation(e_f[:, sl], pre, AF.Exp,
                                     bias=s_bias, scale=1.0)
                nc.scalar.activation(r_f[:, sl], pre, AF.Relu,
                                     bias=s_bias, scale=1.0)
                nc.gpsimd.tensor_scalar_min(e_f[:, sl], e_f[:, sl], 1.0)
                nc.vector.tensor_add(phi_f[:, sl], e_f[:, sl], r_f[:, sl])

            QT = phi_f[:, 0:H]
            KT = phi_f[:, H:2 * H]
            ktok = phip.tile([C, H], BF16)
            Am = [None] * NCH

            def prep_chunks(cs):
                # transposes + A matmuls + masks for chunks cs
                for c in cs:
                    trp = ps_o.tile([C, C], BF16, tag="o")
                    nc.tensor.transpose(trp, KT[:, C * c:C * (c + 1)], s_ident)
                    if c < 4:
                        nc.vector.tensor_copy(ktok[:, C * c:C * (c + 1)], trp)
                    else:
                        nc.scalar.activation(ktok[:, C * c:C * (c + 1)], trp,
                                             AF.Copy)
                for c in cs:
                    A = ps_a.tile([C, C], F32, tag="a")
                    nc.tensor.matmul(A, KT[:, C * c:C * (c + 1)],
                                     QT[:, C * c:C * (c + 1)],
                                     start=True, stop=True)
                    am_c = amp.tile([C, C], BF16, tag="am")
                    Am[c] = am_c
                    nc.vector.tensor_tensor(out=Am[c], in0=A, in1=s_mask, op=MUL)

            # q/k slices for chunks 0-3, then prep, then the rest
            outstage = phip.tile([C, NCH * DV], F32)
            snaps = [None, None]

            phi_slice(0, OFF_QT, 0)
            phi_slice(1, OFF_KT, 0)
            prep_chunks(range(0, 4))

            def run_chunk(c):
                pl = (0, 1) if c == 0 else ((c - 1) % 2,)
                for p in pl:
                    snp = wrk.tile([D, DV + 1], BF16, tag=f"snap{p}")
                    snaps[p] = snp
                    nc.vector.tensor_copy(snaps[p], Sp[p])

                O = ps_o.tile([C, DV + 1], F32, tag="o")
                prev_o = nc.tensor.matmul(O, Am[c], vsl(c), start=True,
                                          stop=False)
                for qi, sn in enumerate(snaps):
                    mm_q = nc.tensor.matmul(O, QT[:, C * c:C * (c + 1)], sn,
                                     start=False, stop=(qi == 1))
                    add_dep_helper(mm_q.ins, prev_o.ins, sync=False,
                                   reason="psum group order")
                    prev_o = mm_q

                mm_su = nc.tensor.matmul(Sp[c % 2], ktok[:, C * c:C * (c + 1)],
                                 vsl(c),
                                 start=False, stop=(c >= NCH - 2),
                                 skip_group_check=True)
                add_dep_helper(mm_su.ins, s_first[c % 2].ins, sync=False,
                               reason="psum group order")
                s_first[c % 2] = mm_su

                rec = wrk.tile([C, 1], F32, tag="rec")
                nc.vector.reciprocal(rec, O[:, DV:DV + 1])
                nc.scalar.activation(outstage[:, DV * c:DV * (c + 1)],
                                     O[:, 0:DV], AF.Copy, bias=0.0, scale=rec)
                if c % 2 == 1 and c < 6:
                    nc.sync.dma_start(
                        out=out[:, DV * (c - 1):DV * (c + 1)],
                        in_=outstage[:, DV * (c - 1):DV * (c + 1)])
                elif c >= 6:
                    nc.sync.dma_start(
                        out=out[:, DV * c:DV * (c + 1)],
                        in_=outstage[:, DV * c:DV * (c + 1)])

            for c in range(4):
                run_chunk(c)
            phi_slice(0, OFF_QT, 1)
            phi_slice(1, OFF_KT, 1)
            prep_chunks(range(4, NCH))
            for c in range(4, NCH):
                run_chunk(c)

    nc.compile()
    return nc


def _get_nc():
    if "nc" not in _cache:
        _cache["nc"] = _build()
    return _cache["nc"]


def _pack_inputs(q, k, v, W_phi, b_phi):
    import ml_dtypes
    bf16 = ml_dtypes.bfloat16

    WT = np.ascontiguousarray(W_phi.T)                    # [d, e]
    maskm = np.triu(np.ones((C, C), np.float32))          # keep tau <= t
    ident = np.eye(C, dtype=np.float32)
    btile = np.tile(b_phi, NCH).reshape(1, H).astype(bf16)

    def aug(vh):  # [H, DV] -> [C, NCH*(DV+1)] partition-major with ones col
        a = np.concatenate([vh, np.ones((H, 1), np.float32)], axis=1)
        return a.reshape(NCH, C, VW).transpose(1, 0, 2).reshape(C, NCH * VW)

    zeros_vp = np.zeros((C, NCH * VW), np.float32)
    zeros_ktp = np.zeros((D, H), np.float32)

    in_maps = []
    for core in range(NCORES):
        b_idx, half = divmod(core, 2)
        sl = slice(half * H, (half + 1) * H)
        b16 = np.empty((D, B16_COLS), np.float32)
        b16[:, OFF_WTB:OFF_WTB + D] = WT
        b16[:, OFF_MASK:OFF_MASK + C] = maskm
        b16[:, OFF_ID:OFF_ID + C] = ident
        b16[:, OFF_BIAS] = b_phi
        b16[:, OFF_QT:OFF_QT + H] = q[b_idx, sl].T
        b16[:, OFF_KT:OFF_KT + H] = k[b_idx, sl].T
        if half == 1:
            b16[:, OFF_KTP:OFF_KTP + H] = k[b_idx, 0:H].T
            b16[:, OFF_VP:OFF_VP + NCH * VW] = aug(v[b_idx, 0:H])
        else:
            b16[:, OFF_KTP:OFF_KTP + H] = zeros_ktp
            b16[:, OFF_VP:OFF_VP + NCH * VW] = zeros_vp
        b16[:, OFF_V:OFF_V + NCH * VW] = aug(v[b_idx, sl])
        in_maps.append({"bin": b16.astype(bf16), "btile": btile})
    return in_maps


def kernel(q, k, v, W_phi, b_phi):
    from concourse.bass_utils import run_bass_kernel_spmd

    q = np.asarray(q, np.float32)
    k = np.asarray(k, np.float32)
    v = np.asarray(v, np.float32)
    W_phi = np.asarray(W_phi, np.float32)
    b_phi = np.asarray(b_phi, np.float32)

    in_maps = _pack_inputs(q, k, v, W_phi, b_phi)
    nc = _get_nc()
    res = run_bass_kernel_spmd(nc, in_maps, list(range(NCORES)))

    out = np.empty((B, T, DV), np.float32)
    for core in range(NCORES):
        b_idx, half = divmod(core, 2)
        o = res.results[core]["out"]                      # [C, NCH*DV]
        o = o.reshape(C, NCH, DV).transpose(1, 0, 2).reshape(H, DV)
        out[b_idx, half * H:(half + 1) * H] = o
    return out


# revision 20
# speedup vs baseline: 1.0310x; 1.0310x over previous
"""Causal linear attention (ELU+1 feature map) on 8 trn2 NeuronCores.

Sharding: core i handles batch b=i//2, sequence half h=i%2 (T=2048 -> 1024
tokens/core).  Second-half cores recompute the first half's running state
S0 = sum_tau phi(k_tau) [v_tau, 1]  (128x129, col 128 = z) from k/v of the
first half; first-half cores get zeroed aux inputs so their S0 == 0.

Math per core (chunk C=128, 8 own chunks + 8 "pre" state-only chunks):
  phi(y) = min(exp(y), 1) + relu(y)            (== ELU(y)+1 exactly)
  A^T_c = K_c Q_c^T ; mask (tau<=t) fused into PSUM->SBUF bf16 copy
  O_c = Amask^T.T @ [V_c, 1] + Q_c @ (Se + So)  (den accumulates in col 128)
  Se/So: parity-split state accumulators (halves the snapshot chain)
  out_c = O_c[:, :128] * (1 / O_c[:, 128])

All inputs in one bf16 pack, DMA'd in need-order pieces across sync(HWDGE)
and Pool(SWDGE); output staged and written in four pieces.
"""

import numpy as np

B, T, D, DV = 4, 2048, 128, 128
H = T // 2          # tokens per core
C = 128             # chunk
NCH = H // C        # chunks per half
NCORES = 8
VW = DV + 1

# bf16 pack columns: [WTb | mask | ident | bias | kTp | qT | kT | vp | v]
OFF_WTB = 0
OFF_MASK = OFF_WTB + D
OFF_ID = OFF_MASK + C
OFF_BIAS = OFF_ID + C
OFF_KTP = OFF_BIAS + 1
OFF_QT = OFF_KTP + H
OFF_KT = OFF_QT + H
OFF_VP = OFF_KT + H
OFF_V = OFF_VP + NCH * VW
B16_COLS = OFF_V + NCH * VW

_cache = {}


def _build():
    import concourse.bacc as bacc
    import concourse.tile as tile
    from concourse import mybir
    from bass_rust import add_dep_helper

    F32 = mybir.dt.float32
    BF16 = mybir.dt.bfloat16
    AF = mybir.ActivationFunctionType
    MUL = mybir.AluOpType.mult

    nc = bacc.Bacc(None, target_bir_lowering=False, debug=False,
                   num_devices=NCORES)

    bin_ = nc.declare_dram_parameter("bin", [D, B16_COLS], BF16, isOutput=False)
    btile = nc.declare_dram_parameter("btile", [1, H], BF16, isOutput=False)
    out = nc.declare_dram_parameter("out", [C, NCH * DV], F32, isOutput=True)

    with tile.TileContext(nc) as tc:
        with (
            tc.tile_pool(name="cst", bufs=1) as cst,
            tc.tile_pool(name="io", bufs=1) as io,
            tc.tile_pool(name="phi", bufs=1) as phip,
            tc.tile_pool(name="am", bufs=NCH) as amp,
            tc.tile_pool(name="wrk", bufs=3) as wrk,
            tc.tile_pool(name="ps_pre", bufs=2, space="PSUM") as ps_pre,
            tc.tile_pool(name="ps_s", bufs=2, space="PSUM") as ps_s,
            tc.tile_pool(name="ps_a", bufs=2, space="PSUM") as ps_a,
            tc.tile_pool(name="ps_o", bufs=2, space="PSUM") as ps_o,
        ):
            # ---- warm the ACT table while DMAs run ----
            s_warm = cst.tile([D, 1], F32)
            nc.vector.memset(s_warm, 0.0)
            s_warm2 = cst.tile([D, 1], BF16)
            nc.scalar.activation(s_warm2, s_warm, AF.Exp)

            # ---- loads: need-ordered pieces across sync + pool ----
            s_b16 = io.tile([D, B16_COLS], BF16)
            s_btile = cst.tile([1, H], BF16)
            s_ones = cst.tile([1, C], BF16)
            nc.sync.dma_start(out=s_btile, in_=btile[:, :])
            nc.vector.memset(s_ones, 1.0)
            nc.sync.dma_start(out=s_b16[:, 0:OFF_KTP], in_=bin_[:, 0:OFF_KTP])
            nc.gpsimd.dma_start(out=s_b16[:, OFF_KTP:OFF_QT],
                                in_=bin_[:, OFF_KTP:OFF_QT])
            nc.sync.dma_start(out=s_b16[:, OFF_QT:OFF_KT],
                              in_=bin_[:, OFF_QT:OFF_KT])
            nc.gpsimd.dma_start(out=s_b16[:, OFF_KT:OFF_VP],
                                in_=bin_[:, OFF_KT:OFF_VP])
            nc.sync.dma_start(out=s_b16[:, OFF_VP:OFF_V],
                              in_=bin_[:, OFF_VP:OFF_V])
            nc.gpsimd.dma_start(out=s_b16[:, OFF_V:B16_COLS],
                                in_=bin_[:, OFF_V:B16_COLS])

            s_bias = s_b16[:, OFF_BIAS:OFF_BIAS + 1]
            sWTb = s_b16[:, OFF_WTB:OFF_WTB + D]
            s_mask = s_b16[:, OFF_MASK:OFF_MASK + C]
            s_ident = s_b16[:, OFF_ID:OFF_ID + C]

            def vsl(c):
                return s_b16[:, OFF_V + VW * c:OFF_V + VW * (c + 1)]

            def vpsl(c):
                return s_b16[:, OFF_VP + VW * c:OFF_VP + VW * (c + 1)]

            # parity-split state accumulators [D, DV+1]
            Se = ps_s.tile([D, DV + 1], F32, tag="s")
            So = ps_s.tile([D, DV + 1], F32, tag="s")
            Sp = [Se, So]
            started = [False, False]
            s_first = [None, None]

            # ---- token-major pre for K_pre (state recompute path first) ----
            phi_t = phip.tile([C, H], BF16)       # K_tok_pre chunks
            e_t = phip.tile([C, H], BF16)
            r_t = phip.tile([C, H], BF16)
            for j in range(H // 512):
                pst = ps_pre.tile([C, 512], F32, tag="pre")
                prev = nc.tensor.matmul(pst, s_ones,
                                 s_btile[:, 512 * j:512 * (j + 1)],
                                 start=True, stop=False)
                for cc in range(4):
                    c = 4 * j + cc
                    mm_c = nc.tensor.matmul(pst[:, C * cc:C * (cc + 1)],
                                     s_b16[:, OFF_KTP + C * c:OFF_KTP + C * (c + 1)],
                                     sWTb, start=False, stop=(cc == 3))
                    add_dep_helper(mm_c.ins, prev.ins, sync=False,
                                   reason="psum group order")
                    prev = mm_c
                sl = slice(512 * j, 512 * (j + 1))
                nc.scalar.activation(e_t[:, sl], pst, AF.Exp)
                nc.vector.tensor_scalar_max(r_t[:, sl], pst, 0.0)
                nc.gpsimd.tensor_scalar_min(e_t[:, sl], e_t[:, sl], 1.0)
                nc.vector.tensor_add(phi_t[:, sl], e_t[:, sl], r_t[:, sl])
                # pre-half state contributions (zeros on half-0 cores)
                for cc in range(4):
                    c = 4 * j + cc
                    p = c % 2
                    mm_s = nc.tensor.matmul(Sp[p], phi_t[:, C * c:C * (c + 1)],
                                     vpsl(c),
                                     start=(not started[p]), stop=False,
                                     skip_group_check=True)
                    if started[p]:
                        add_dep_helper(mm_s.ins, s_first[p].ins, sync=False,
                                       reason="psum group order")
                    s_first[p] = mm_s
                    started[p] = True

            # ---- feature-major phi for own q, k (512-col pipeline) ----
            phi_f = phip.tile([D, 2 * H], BF16)   # [Q^T | K^T]
            e_f = phip.tile([D, 2 * H], BF16)
            r_f = phip.tile([D, 2 * H], BF16)

            def phi_slice(i, off, j):
                pre = ps_pre.tile([D, 512], F32, tag="pre")
                nc.tensor.matmul(pre, sWTb,
                                 s_b16[:, off + 512 * j:off + 512 * (j + 1)],
                                 start=True, stop=True)
                sl = slice(H * i + 512 * j, H * i + 512 * (j + 1))
                nc.scalar.activation(e_f[:, sl], pre, AF.Exp,
                                     bias=s_bias, scale=1.0)
                nc.scalar.activation(r_f[:, sl], pre, AF.Relu,
                                     bias=s_bias, scale=1.0)
                nc.gpsimd.tensor_scalar_min(e_f[:, sl], e_f[:, sl], 1.0)
                nc.vector.tensor_add(phi_f[:, sl], e_f[:, sl], r_f[:, sl])

            QT = phi_f[:, 0:H]
            KT = phi_f[:, H:2 * H]
            ktok = phip.tile([C, H], BF16)
            Am = [None] * NCH

            def prep_chunks(cs):
                # transposes + A matmuls + masks for chunks cs
                for c in cs:
                    trp = ps_o.tile([C, C], BF16, tag="o")
                    nc.tensor.transpose(trp, KT[:, C * c:C * (c + 1)], s_ident)
                    if c < 4:
                        nc.vector.tensor_copy(ktok[:, C * c:C * (c + 1)], trp)
                    else:
                        nc.scalar.activation(ktok[:, C * c:C * (c + 1)], trp,
                                             AF.Copy)
                for c in cs:
                    A = ps_a.tile([C, C], F32, tag="a")
                    nc.tensor.matmul(A, KT[:, C * c:C * (c + 1)],
                                     QT[:, C * c:C * (c + 1)],
                                     start=True, stop=True)
                    am_c = amp.tile([C, C], BF16, tag="am")
                    Am[c] = am_c
                    nc.vector.tensor_tensor(out=Am[c], in0=A, in1=s_mask, op=MUL)

            # q/k slices for chunks 0-3, then prep, then the rest
            outstage = phip.tile([C, NCH * DV], F32)
            snaps = [None, None]

            phi_slice(0, OFF_QT, 0)
            phi_slice(1, OFF_KT, 0)
            prep_chunks(range(0, 4))

            def run_chunk(c):
                pl = (0, 1) if c == 0 else ((c - 1) % 2,)
                for p in pl:
                    snp = wrk.tile([D, DV + 1], BF16, tag=f"snap{p}")
                    snaps[p] = snp
                    nc.vector.tensor_copy(snaps[p], Sp[p])

                O = ps_o.tile([C, DV + 1], F32, tag="o")
                prev_o = nc.tensor.matmul(O, Am[c], vsl(c), start=True,
                                          stop=False)
                for qi, sn in enumerate(snaps):
                    mm_q = nc.tensor.matmul(O, QT[:, C * c:C * (c + 1)], sn,
                                     start=False, stop=(qi == 1))
                    add_dep_helper(mm_q.ins, prev_o.ins, sync=False,
                                   reason="psum group order")
                    prev_o = mm_q

                mm_su = nc.tensor.matmul(Sp[c % 2], ktok[:, C * c:C * (c + 1)],
                                 vsl(c),
                                 start=False, stop=(c >= NCH - 2),
                                 skip_group_check=True)
                add_dep_helper(mm_su.ins, s_first[c % 2].ins, sync=False,
                               reason="psum group order")
                s_first[c % 2] = mm_su

                rec = wrk.tile([C, 1], F32, tag="rec")
                nc.vector.reciprocal(rec, O[:, DV:DV + 1])
                nc.scalar.activation(outstage[:, DV * c:DV * (c + 1)],
                                     O[:, 0:DV], AF.Copy, bias=0.0, scale=rec)
                if c % 2 == 1 and c < 6:
                    nc.sync.dma_start(
                        out=out[:, DV * (c - 1):DV * (c + 1)],
                        in_=outstage[:, DV * (c - 1):DV * (c + 1)])
                elif c >= 6:
                    nc.sync.dma_start(
                        out=out[:, DV * c:DV * (c + 1)],
                        in_=outstage[:, DV * c:DV * (c + 1)])

            for c in range(4):
                run_chunk(c)
            phi_slice(0, OFF_QT, 1)
            phi_slice(1, OFF_KT, 1)
            prep_chunks(range(4, NCH))
            for c in range(4, NCH):
                run_chunk(c)

    nc.compile()
    return nc


def _get_nc():
    if "nc" not in _cache:
        _cache["nc"] = _build()
    return _cache["nc"]


def _pack_inputs(q, k, v, W_phi, b_phi):
    import ml_dtypes
    bf16 = ml_dtypes.bfloat16

    WT = np.ascontiguousarray(W_phi.T)                    # [d, e]
    maskm = np.triu(np.ones((C, C), np.float32))          # keep tau <= t
    ident = np.eye(C, dtype=np.float32)
    btile = np.tile(b_phi, NCH).reshape(1, H).astype(bf16)

    def aug(vh):  # [H, DV] -> [C, NCH*(DV+1)] partition-major with ones col
        a = np.concatenate([vh, np.ones((H, 1), np.float32)], axis=1)
        return a.reshape(NCH, C, VW).transpose(1, 0, 2).reshape(C, NCH * VW)

    zeros_vp = np.zeros((C, NCH * VW), np.float32)
    zeros_ktp = np.zeros((D, H), np.float32)

    in_maps = []
    for core in range(NCORES):
        b_idx, half = divmod(core, 2)
        sl = slice(half * H, (half + 1) * H)
        b16 = np.empty((D, B16_COLS), np.float32)
        b16[:, OFF_WTB:OFF_WTB + D] = WT
        b16[:, OFF_MASK:OFF_MASK + C] = maskm
        b16[:, OFF_ID:OFF_ID + C] = ident
        b16[:, OFF_BIAS] = b_phi
        b16[:, OFF_QT:OFF_QT + H] = q[b_idx, sl].T
        b16[:, OFF_KT:OFF_KT + H] = k[b_idx, sl].T
        if half == 1:
            b16[:, OFF_KTP:OFF_KTP + H] = k[b_idx, 0:H].T
            b16[:, OFF_VP:OFF_VP + NCH * VW] = aug(v[b_idx, 0:H])
        else:
            b16[:, OFF_KTP:OFF_KTP + H] = zeros_ktp
            b16[:, OFF_VP:OFF_VP + NCH * VW] = zeros_vp
        b16[:, OFF_V:OFF_V + NCH * VW] = aug(v[b_idx, sl])
        in_maps.append({"bin": b16.astype(bf16), "btile": btile})
    return in_maps


def kernel(q, k, v, W_phi, b_phi):
    from concourse.bass_utils import run_bass_kernel_spmd

    q = np.asarray(q, np.float32)
    k = np.asarray(k, np.float32)
    v = np.asarray(v, np.float32)
    W_phi = np.asarray(W_phi, np.float32)
    b_phi = np.asarray(b_phi, np.float32)

    in_maps = _pack_inputs(q, k, v, W_phi, b_phi)
    nc = _get_nc()
    res = run_bass_kernel_spmd(nc, in_maps, list(range(NCORES)))

    out = np.empty((B, T, DV), np.float32)
    for core in range(NCORES):
        b_idx, half = divmod(core, 2)
        o = res.results[core]["out"]                      # [C, NCH*DV]
        o = o.reshape(C, NCH, DV).transpose(1, 0, 2).reshape(H, DV)
        out[b_idx, half * H:(half + 1) * H] = o
    return out


# revision 21
# speedup vs baseline: 1.0504x; 1.0188x over previous
"""Causal linear attention (ELU+1 feature map) on 8 trn2 NeuronCores.

Sharding: core i handles batch b=i//2, sequence half h=i%2 (T=2048 -> 1024
tokens/core).  Second-half cores recompute the first half's running state
S0 = sum_tau phi(k_tau) [v_tau, 1]  (128x129, col 128 = z) from k/v of the
first half; first-half cores get zeroed aux inputs so their S0 == 0.

Math per core (chunk C=128, 8 own chunks + 8 "pre" state-only chunks):
  phi(y) = min(exp(y), 1) + relu(y)            (== ELU(y)+1 exactly)
  A^T_c = K_c Q_c^T ; mask (tau<=t) fused into PSUM->SBUF bf16 copy
  O_c = Amask^T.T @ [V_c, 1] + Q_c @ (Se + So)  (den accumulates in col 128)
  Se/So: parity-split state accumulators (halves the snapshot chain)
  out_c = O_c[:, :128] * (1 / O_c[:, 128])

All inputs in one bf16 pack, DMA'd in need-order pieces across sync(HWDGE)
and Pool(SWDGE); output staged and written in four pieces.
"""

import numpy as np

B, T, D, DV = 4, 2048, 128, 128
H = T // 2          # tokens per core
C = 128             # chunk
NCH = H // C        # chunks per half
NCORES = 8
VW = DV + 1

# bf16 pack columns: [WTb | mask | ident | bias | kTp | qT | kT | vp | v]
OFF_WTB = 0
OFF_MASK = OFF_WTB + D
OFF_ID = OFF_MASK + C
OFF_BIAS = OFF_ID + C
OFF_KTP = OFF_BIAS + 1
OFF_QT = OFF_KTP + H
OFF_KT = OFF_QT + H
OFF_VP = OFF_KT + H
OFF_V = OFF_VP + NCH * VW
B16_COLS = OFF_V + NCH * VW

_cache = {}


def _build():
    import concourse.bacc as bacc
    import concourse.tile as tile
    from concourse import mybir
    from bass_rust import add_dep_helper

    F32 = mybir.dt.float32
    BF16 = mybir.dt.bfloat16
    AF = mybir.ActivationFunctionType
    MUL = mybir.AluOpType.mult

    nc = bacc.Bacc(None, target_bir_lowering=False, debug=False,
                   num_devices=NCORES)

    bin_ = nc.declare_dram_parameter("bin", [D, B16_COLS], BF16, isOutput=False)
    btile = nc.declare_dram_parameter("btile", [1, H], BF16, isOutput=False)
    out = nc.declare_dram_parameter("out", [C, NCH * DV], F32, isOutput=True)

    with tile.TileContext(nc) as tc:
        with (
            tc.tile_pool(name="cst", bufs=1) as cst,
            tc.tile_pool(name="io", bufs=1) as io,
            tc.tile_pool(name="phi", bufs=1) as phip,
            tc.tile_pool(name="am", bufs=NCH) as amp,
            tc.tile_pool(name="wrk", bufs=3) as wrk,
            tc.tile_pool(name="ps_pre", bufs=2, space="PSUM") as ps_pre,
            tc.tile_pool(name="ps_s", bufs=2, space="PSUM") as ps_s,
            tc.tile_pool(name="ps_a", bufs=2, space="PSUM") as ps_a,
            tc.tile_pool(name="ps_o", bufs=2, space="PSUM") as ps_o,
        ):
            # ---- warm the ACT table while DMAs run ----
            s_warm = cst.tile([D, 1], F32)
            nc.vector.memset(s_warm, 0.0)
            s_warm2 = cst.tile([D, 1], BF16)
            nc.scalar.activation(s_warm2, s_warm, AF.Exp)

            # ---- loads: need-ordered pieces across sync + pool ----
            s_b16 = io.tile([D, B16_COLS], BF16)
            s_btile = cst.tile([1, H], BF16)
            s_ones = cst.tile([1, C], BF16)
            nc.sync.dma_start(out=s_btile, in_=btile[:, :])
            nc.vector.memset(s_ones, 1.0)
            nc.sync.dma_start(out=s_b16[:, 0:OFF_KTP], in_=bin_[:, 0:OFF_KTP])
            nc.gpsimd.dma_start(out=s_b16[:, OFF_KTP:OFF_QT],
                                in_=bin_[:, OFF_KTP:OFF_QT])
            nc.sync.dma_start(out=s_b16[:, OFF_QT:OFF_KT],
                              in_=bin_[:, OFF_QT:OFF_KT])
            nc.gpsimd.dma_start(out=s_b16[:, OFF_KT:OFF_VP],
                                in_=bin_[:, OFF_KT:OFF_VP])
            nc.sync.dma_start(out=s_b16[:, OFF_VP:OFF_V],
                              in_=bin_[:, OFF_VP:OFF_V])
            nc.gpsimd.dma_start(out=s_b16[:, OFF_V:B16_COLS],
                                in_=bin_[:, OFF_V:B16_COLS])

            s_bias = s_b16[:, OFF_BIAS:OFF_BIAS + 1]
            sWTb = s_b16[:, OFF_WTB:OFF_WTB + D]
            s_mask = s_b16[:, OFF_MASK:OFF_MASK + C]
            s_ident = s_b16[:, OFF_ID:OFF_ID + C]

            def vsl(c):
                return s_b16[:, OFF_V + VW * c:OFF_V + VW * (c + 1)]

            def vpsl(c):
                return s_b16[:, OFF_VP + VW * c:OFF_VP + VW * (c + 1)]

            # parity-split state accumulators [D, DV+1]
            Se = ps_s.tile([D, DV + 1], F32, tag="s")
            So = ps_s.tile([D, DV + 1], F32, tag="s")
            Sp = [Se, So]
            started = [False, False]
            s_first = [None, None]

            # ---- token-major pre for K_pre (state recompute path first) ----
            phi_t = phip.tile([C, H], BF16)       # K_tok_pre chunks
            e_t = phip.tile([C, H], BF16)
            r_t = phip.tile([C, H], BF16)
            for j in range(H // 512):
                pst = ps_pre.tile([C, 512], F32, tag="pre")
                prev = nc.tensor.matmul(pst, s_ones,
                                 s_btile[:, 512 * j:512 * (j + 1)],
                                 start=True, stop=False)
                for cc in range(4):
                    c = 4 * j + cc
                    mm_c = nc.tensor.matmul(pst[:, C * cc:C * (cc + 1)],
                                     s_b16[:, OFF_KTP + C * c:OFF_KTP + C * (c + 1)],
                                     sWTb, start=False, stop=(cc == 3))
                    add_dep_helper(mm_c.ins, prev.ins, sync=False,
                                   reason="psum group order")
                    prev = mm_c
                sl = slice(512 * j, 512 * (j + 1))
                nc.scalar.activation(e_t[:, sl], pst, AF.Exp)
                nc.vector.tensor_scalar_max(r_t[:, sl], pst, 0.0)
                nc.gpsimd.tensor_scalar_min(e_t[:, sl], e_t[:, sl], 1.0)
                nc.vector.tensor_add(phi_t[:, sl], e_t[:, sl], r_t[:, sl])
                # pre-half state contributions (zeros on half-0 cores)
                for cc in range(4):
                    c = 4 * j + cc
                    p = c % 2
                    mm_s = nc.tensor.matmul(Sp[p], phi_t[:, C * c:C * (c + 1)],
                                     vpsl(c),
                                     start=(not started[p]), stop=False,
                                     skip_group_check=True)
                    if started[p]:
                        add_dep_helper(mm_s.ins, s_first[p].ins, sync=False,
                                       reason="psum group order")
                    s_first[p] = mm_s
                    started[p] = True

            # ---- feature-major phi for own q, k (512-col pipeline) ----
            phi_f = phip.tile([D, 2 * H], BF16)   # [Q^T | K^T]
            e_f = phip.tile([D, 2 * H], BF16)
            r_f = phip.tile([D, 2 * H], BF16)

            def phi_slice(i, off, j):
                pre = ps_pre.tile([D, 512], F32, tag="pre")
                nc.tensor.matmul(pre, sWTb,
                                 s_b16[:, off + 512 * j:off + 512 * (j + 1)],
                                 start=True, stop=True)
                sl = slice(H * i + 512 * j, H * i + 512 * (j + 1))
                nc.scalar.activation(e_f[:, sl], pre, AF.Exp,
                                     bias=s_bias, scale=1.0)
                nc.scalar.activation(r_f[:, sl], pre, AF.Relu,
                                     bias=s_bias, scale=1.0)
                nc.gpsimd.tensor_scalar_min(e_f[:, sl], e_f[:, sl], 1.0)
                nc.vector.tensor_add(phi_f[:, sl], e_f[:, sl], r_f[:, sl])

            QT = phi_f[:, 0:H]
            KT = phi_f[:, H:2 * H]
            ktok = phip.tile([C, H], BF16)
            Am = [None] * NCH

            def prep_chunks(cs):
                # transposes + A matmuls + masks for chunks cs
                for c in cs:
                    trp = ps_o.tile([C, C], BF16, tag="o")
                    nc.tensor.transpose(trp, KT[:, C * c:C * (c + 1)], s_ident)
                    if c < 4:
                        nc.vector.tensor_copy(ktok[:, C * c:C * (c + 1)], trp)
                    else:
                        nc.scalar.activation(ktok[:, C * c:C * (c + 1)], trp,
                                             AF.Copy)
                for c in cs:
                    A = ps_a.tile([C, C], F32, tag="a")
                    nc.tensor.matmul(A, KT[:, C * c:C * (c + 1)],
                                     QT[:, C * c:C * (c + 1)],
                                     start=True, stop=True)
                    am_c = amp.tile([C, C], BF16, tag="am")
                    Am[c] = am_c
                    nc.vector.tensor_tensor(out=Am[c], in0=A, in1=s_mask, op=MUL)

            # q/k slices for chunks 0-3, then prep, then the rest
            outstage = phip.tile([C, NCH * DV], F32)
            snaps = [None, None]

            phi_slice(0, OFF_QT, 0)
            phi_slice(1, OFF_KT, 0)
            prep_chunks(range(0, 4))

            def run_chunk(c):
                pl = (0, 1) if c == 0 else ((c - 1) % 2,)
                for p in pl:
                    snp = wrk.tile([D, DV + 1], BF16, tag=f"snap{p}")
                    snaps[p] = snp
                    nc.vector.tensor_copy(snaps[p], Sp[p])

                O = ps_o.tile([C, DV + 1], F32, tag="o")
                prev_o = nc.tensor.matmul(O, Am[c], vsl(c), start=True,
                                          stop=False)
                for qi, sn in enumerate(snaps):
                    mm_q = nc.tensor.matmul(O, QT[:, C * c:C * (c + 1)], sn,
                                     start=False, stop=(qi == 1))
                    add_dep_helper(mm_q.ins, prev_o.ins, sync=False,
                                   reason="psum group order")
                    prev_o = mm_q

                mm_su = nc.tensor.matmul(Sp[c % 2], ktok[:, C * c:C * (c + 1)],
                                 vsl(c),
                                 start=False, stop=(c >= NCH - 2),
                                 skip_group_check=True)
                add_dep_helper(mm_su.ins, s_first[c % 2].ins, sync=False,
                               reason="psum group order")
                s_first[c % 2] = mm_su

                rec = wrk.tile([C, 1], F32, tag="rec")
                nc.vector.reciprocal(rec, O[:, DV:DV + 1])
                nc.scalar.activation(outstage[:, DV * c:DV * (c + 1)],
                                     O[:, 0:DV], AF.Copy, bias=0.0, scale=rec)
                if c % 2 == 1:
                    nc.sync.dma_start(
                        out=out[:, DV * (c - 1):DV * (c + 1)],
                        in_=outstage[:, DV * (c - 1):DV * (c + 1)])

            for c in range(4):
                run_chunk(c)
            phi_slice(0, OFF_QT, 1)
            phi_slice(1, OFF_KT, 1)
            prep_chunks(range(4, NCH))
            for c in range(4, NCH):
                run_chunk(c)

    nc.compile()
    return nc


def _get_nc():
    if "nc" not in _cache:
        _cache["nc"] = _build()
    return _cache["nc"]


def _pack_inputs(q, k, v, W_phi, b_phi):
    import ml_dtypes
    bf16 = ml_dtypes.bfloat16

    WT = np.ascontiguousarray(W_phi.T)                    # [d, e]
    maskm = np.triu(np.ones((C, C), np.float32))          # keep tau <= t
    ident = np.eye(C, dtype=np.float32)
    btile = np.tile(b_phi, NCH).reshape(1, H).astype(bf16)

    def aug(vh):  # [H, DV] -> [C, NCH*(DV+1)] partition-major with ones col
        a = np.concatenate([vh, np.ones((H, 1), np.float32)], axis=1)
        return a.reshape(NCH, C, VW).transpose(1, 0, 2).reshape(C, NCH * VW)

    zeros_vp = np.zeros((C, NCH * VW), np.float32)
    zeros_ktp = np.zeros((D, H), np.float32)

    in_maps = []
    for core in range(NCORES):
        b_idx, half = divmod(core, 2)
        sl = slice(half * H, (half + 1) * H)
        b16 = np.empty((D, B16_COLS), np.float32)
        b16[:, OFF_WTB:OFF_WTB + D] = WT
        b16[:, OFF_MASK:OFF_MASK + C] = maskm
        b16[:, OFF_ID:OFF_ID + C] = ident
        b16[:, OFF_BIAS] = b_phi
        b16[:, OFF_QT:OFF_QT + H] = q[b_idx, sl].T
        b16[:, OFF_KT:OFF_KT + H] = k[b_idx, sl].T
        if half == 1:
            b16[:, OFF_KTP:OFF_KTP + H] = k[b_idx, 0:H].T
            b16[:, OFF_VP:OFF_VP + NCH * VW] = aug(v[b_idx, 0:H])
        else:
            b16[:, OFF_KTP:OFF_KTP + H] = zeros_ktp
            b16[:, OFF_VP:OFF_VP + NCH * VW] = zeros_vp
        b16[:, OFF_V:OFF_V + NCH * VW] = aug(v[b_idx, sl])
        in_maps.append({"bin": b16.astype(bf16), "btile": btile})
    return in_maps


def kernel(q, k, v, W_phi, b_phi):
    from concourse.bass_utils import run_bass_kernel_spmd

    q = np.asarray(q, np.float32)
    k = np.asarray(k, np.float32)
    v = np.asarray(v, np.float32)
    W_phi = np.asarray(W_phi, np.float32)
    b_phi = np.asarray(b_phi, np.float32)

    in_maps = _pack_inputs(q, k, v, W_phi, b_phi)
    nc = _get_nc()
    res = run_bass_kernel_spmd(nc, in_maps, list(range(NCORES)))

    out = np.empty((B, T, DV), np.float32)
    for core in range(NCORES):
        b_idx, half = divmod(core, 2)
        o = res.results[core]["out"]                      # [C, NCH*DV]
        o = o.reshape(C, NCH, DV).transpose(1, 0, 2).reshape(H, DV)
        out[b_idx, half * H:(half + 1) * H] = o
    return out


# revision 22
# speedup vs baseline: 1.0654x; 1.0144x over previous
"""Causal linear attention (ELU+1 feature map) on 8 trn2 NeuronCores.

Sharding: core i handles batch b=i//2, sequence half h=i%2 (T=2048 -> 1024
tokens/core).  Second-half cores recompute the first half's running state
S0 = sum_tau phi(k_tau) [v_tau, 1]  (128x129, col 128 = z) from k/v of the
first half; first-half cores get zeroed aux inputs so their S0 == 0.

Math per core (chunk C=128, 8 own chunks + 8 "pre" state-only chunks):
  phi(y) = min(exp(y), 1) + relu(y)            (== ELU(y)+1 exactly)
  A^T_c = K_c Q_c^T ; mask (tau<=t) fused into PSUM->SBUF bf16 copy
  O_c = Amask^T.T @ [V_c, 1] + Q_c @ (Se + So)  (den accumulates in col 128)
  Se/So: parity-split state accumulators (halves the snapshot chain)
  out_c = O_c[:, :128] * (1 / O_c[:, 128])

All inputs in one bf16 pack, DMA'd in need-order pieces across sync(HWDGE)
and Pool(SWDGE); output staged and written in four pieces.
"""

import numpy as np

B, T, D, DV = 4, 2048, 128, 128
H = T // 2          # tokens per core
C = 128             # chunk
NCH = H // C        # chunks per half
NCORES = 8
VW = DV + 1

# bf16 pack columns: [WTb | mask | ident | bias | kTp | qT | kT | vp | v]
OFF_WTB = 0
OFF_MASK = OFF_WTB + D
OFF_ID = OFF_MASK + C
OFF_BIAS = OFF_ID + C
OFF_KTP = OFF_BIAS + 1
OFF_QT = OFF_KTP + H
OFF_KT = OFF_QT + H
OFF_VP = OFF_KT + H
OFF_V = OFF_VP + NCH * VW
B16_COLS = OFF_V + NCH * VW

_cache = {}


def _build():
    import concourse.bacc as bacc
    import concourse.tile as tile
    from concourse import mybir
    from bass_rust import add_dep_helper

    F32 = mybir.dt.float32
    BF16 = mybir.dt.bfloat16
    AF = mybir.ActivationFunctionType
    MUL = mybir.AluOpType.mult

    nc = bacc.Bacc(None, target_bir_lowering=False, debug=False,
                   num_devices=NCORES)

    bin_ = nc.declare_dram_parameter("bin", [D, B16_COLS], BF16, isOutput=False)
    btile = nc.declare_dram_parameter("btile", [1, H], BF16, isOutput=False)
    out = nc.declare_dram_parameter("out", [C, NCH * DV], F32, isOutput=True)

    with tile.TileContext(nc) as tc:
        with (
            tc.tile_pool(name="cst", bufs=1) as cst,
            tc.tile_pool(name="io", bufs=1) as io,
            tc.tile_pool(name="phi", bufs=1) as phip,
            tc.tile_pool(name="am", bufs=NCH) as amp,
            tc.tile_pool(name="wrk", bufs=3) as wrk,
            tc.tile_pool(name="ps_pre", bufs=2, space="PSUM") as ps_pre,
            tc.tile_pool(name="ps_s", bufs=2, space="PSUM") as ps_s,
            tc.tile_pool(name="ps_a", bufs=2, space="PSUM") as ps_a,
            tc.tile_pool(name="ps_o", bufs=2, space="PSUM") as ps_o,
        ):
            # ---- warm the ACT table while DMAs run ----
            s_warm = cst.tile([D, 1], F32)
            nc.vector.memset(s_warm, 0.0)
            s_warm2 = cst.tile([D, 1], BF16)
            nc.scalar.activation(s_warm2, s_warm, AF.Exp)

            # ---- loads: need-ordered pieces across sync + pool ----
            s_b16 = io.tile([D, B16_COLS], BF16)
            s_btile = cst.tile([1, H], BF16)
            s_ones = cst.tile([1, C], BF16)
            nc.sync.dma_start(out=s_btile, in_=btile[:, :])
            nc.vector.memset(s_ones, 1.0)
            nc.sync.dma_start(out=s_b16[:, 0:OFF_KTP], in_=bin_[:, 0:OFF_KTP])
            nc.gpsimd.dma_start(out=s_b16[:, OFF_KTP:OFF_QT],
                                in_=bin_[:, OFF_KTP:OFF_QT])
            nc.sync.dma_start(out=s_b16[:, OFF_QT:OFF_KT],
                              in_=bin_[:, OFF_QT:OFF_KT])
            nc.gpsimd.dma_start(out=s_b16[:, OFF_KT:OFF_VP],
                                in_=bin_[:, OFF_KT:OFF_VP])
            nc.sync.dma_start(out=s_b16[:, OFF_VP:OFF_V],
                              in_=bin_[:, OFF_VP:OFF_V])
            nc.gpsimd.dma_start(out=s_b16[:, OFF_V:B16_COLS],
                                in_=bin_[:, OFF_V:B16_COLS])

            s_bias = s_b16[:, OFF_BIAS:OFF_BIAS + 1]
            sWTb = s_b16[:, OFF_WTB:OFF_WTB + D]
            s_mask = s_b16[:, OFF_MASK:OFF_MASK + C]
            s_ident = s_b16[:, OFF_ID:OFF_ID + C]

            def vsl(c):
                return s_b16[:, OFF_V + VW * c:OFF_V + VW * (c + 1)]

            def vpsl(c):
                return s_b16[:, OFF_VP + VW * c:OFF_VP + VW * (c + 1)]

            # parity-split state accumulators [D, DV+1]
            Se = ps_s.tile([D, DV + 1], F32, tag="s")
            So = ps_s.tile([D, DV + 1], F32, tag="s")
            Sp = [Se, So]
            started = [False, False]
            s_first = [None, None]

            # ---- token-major pre for K_pre (state recompute path first) ----
            phi_t = phip.tile([C, H], BF16)       # K_tok_pre chunks
            e_t = phip.tile([C, H], BF16)
            r_t = phip.tile([C, H], BF16)
            for j in range(H // 512):
                pst = ps_pre.tile([C, 512], F32, tag="pre")
                prev = nc.tensor.matmul(pst, s_ones,
                                 s_btile[:, 512 * j:512 * (j + 1)],
                                 start=True, stop=False)
                for cc in range(4):
                    c = 4 * j + cc
                    mm_c = nc.tensor.matmul(pst[:, C * cc:C * (cc + 1)],
                                     s_b16[:, OFF_KTP + C * c:OFF_KTP + C * (c + 1)],
                                     sWTb, start=False, stop=(cc == 3))
                    add_dep_helper(mm_c.ins, prev.ins, sync=False,
                                   reason="psum group order")
                    prev = mm_c
                sl = slice(512 * j, 512 * (j + 1))
                nc.scalar.activation(e_t[:, sl], pst, AF.Exp)
                nc.vector.tensor_scalar_max(r_t[:, sl], pst, 0.0)
                nc.gpsimd.tensor_scalar_min(e_t[:, sl], e_t[:, sl], 1.0)
                nc.vector.tensor_add(phi_t[:, sl], e_t[:, sl], r_t[:, sl])
                # pre-half state contributions (zeros on half-0 cores)
                for cc in range(4):
                    c = 4 * j + cc
                    p = c % 2
                    mm_s = nc.tensor.matmul(Sp[p], phi_t[:, C * c:C * (c + 1)],
                                     vpsl(c),
                                     start=(not started[p]), stop=False,
                                     skip_group_check=True)
                    if started[p]:
                        add_dep_helper(mm_s.ins, s_first[p].ins, sync=False,
                                       reason="psum group order")
                    s_first[p] = mm_s
                    started[p] = True

            # ---- feature-major phi for own q, k (512-col pipeline) ----
            phi_f = phip.tile([D, 2 * H], BF16)   # [Q^T | K^T]
            e_f = phip.tile([D, 2 * H], BF16)
            r_f = phip.tile([D, 2 * H], BF16)

            def phi_slice(i, off, j):
                pre = ps_pre.tile([D, 512], F32, tag="pre")
                nc.tensor.matmul(pre, sWTb,
                                 s_b16[:, off + 512 * j:off + 512 * (j + 1)],
                                 start=True, stop=True)
                sl = slice(H * i + 512 * j, H * i + 512 * (j + 1))
                nc.scalar.activation(e_f[:, sl], pre, AF.Exp,
                                     bias=s_bias, scale=1.0)
                nc.scalar.activation(r_f[:, sl], pre, AF.Relu,
                                     bias=s_bias, scale=1.0)
                nc.gpsimd.tensor_scalar_min(e_f[:, sl], e_f[:, sl], 1.0)
                nc.vector.tensor_add(phi_f[:, sl], e_f[:, sl], r_f[:, sl])

            QT = phi_f[:, 0:H]
            KT = phi_f[:, H:2 * H]
            ktok = phip.tile([C, H], BF16)
            Am = [None] * NCH

            def prep_chunks(cs):
                # transposes + A matmuls + masks for chunks cs
                for c in cs:
                    trp = ps_o.tile([C, C], BF16, tag="o")
                    nc.tensor.transpose(trp, KT[:, C * c:C * (c + 1)], s_ident)
                    if c % 2 == 0:
                        nc.vector.tensor_copy(ktok[:, C * c:C * (c + 1)], trp)
                    else:
                        nc.scalar.activation(ktok[:, C * c:C * (c + 1)], trp,
                                             AF.Copy)
                for c in cs:
                    A = ps_a.tile([C, C], F32, tag="a")
                    nc.tensor.matmul(A, KT[:, C * c:C * (c + 1)],
                                     QT[:, C * c:C * (c + 1)],
                                     start=True, stop=True)
                    am_c = amp.tile([C, C], BF16, tag="am")
                    Am[c] = am_c
                    nc.vector.tensor_tensor(out=Am[c], in0=A, in1=s_mask, op=MUL)

            # q/k slices for chunks 0-3, then prep, then the rest
            outstage = phip.tile([C, NCH * DV], F32)
            snaps = [None, None]

            phi_slice(0, OFF_QT, 0)
            phi_slice(1, OFF_KT, 0)
            prep_chunks(range(0, 4))

            def run_chunk(c):
                pl = (0, 1) if c == 0 else ((c - 1) % 2,)
                for p in pl:
                    snp = wrk.tile([D, DV + 1], BF16, tag=f"snap{p}")
                    snaps[p] = snp
                    nc.vector.tensor_copy(snaps[p], Sp[p])

                O = ps_o.tile([C, DV + 1], F32, tag="o")
                prev_o = nc.tensor.matmul(O, Am[c], vsl(c), start=True,
                                          stop=False)
                for qi, sn in enumerate(snaps):
                    mm_q = nc.tensor.matmul(O, QT[:, C * c:C * (c + 1)], sn,
                                     start=False, stop=(qi == 1))
                    add_dep_helper(mm_q.ins, prev_o.ins, sync=False,
                                   reason="psum group order")
                    prev_o = mm_q

                mm_su = nc.tensor.matmul(Sp[c % 2], ktok[:, C * c:C * (c + 1)],
                                 vsl(c),
                                 start=False, stop=(c >= NCH - 2),
                                 skip_group_check=True)
                add_dep_helper(mm_su.ins, s_first[c % 2].ins, sync=False,
                               reason="psum group order")
                s_first[c % 2] = mm_su

                rec = wrk.tile([C, 1], F32, tag="rec")
                nc.vector.reciprocal(rec, O[:, DV:DV + 1])
                nc.scalar.activation(outstage[:, DV * c:DV * (c + 1)],
                                     O[:, 0:DV], AF.Copy, bias=0.0, scale=rec)
                if c % 2 == 1:
                    nc.sync.dma_start(
                        out=out[:, DV * (c - 1):DV * (c + 1)],
                        in_=outstage[:, DV * (c - 1):DV * (c + 1)])

            for c in range(4):
                run_chunk(c)
            phi_slice(0, OFF_QT, 1)
            phi_slice(1, OFF_KT, 1)
            prep_chunks(range(4, NCH))
            for c in range(4, NCH):
                run_chunk(c)

    nc.compile()
    return nc


def _get_nc():
    if "nc" not in _cache:
        _cache["nc"] = _build()
    return _cache["nc"]


def _pack_inputs(q, k, v, W_phi, b_phi):
    import ml_dtypes
    bf16 = ml_dtypes.bfloat16

    WT = np.ascontiguousarray(W_phi.T)                    # [d, e]
    maskm = np.triu(np.ones((C, C), np.float32))          # keep tau <= t
    ident = np.eye(C, dtype=np.float32)
    btile = np.tile(b_phi, NCH).reshape(1, H).astype(bf16)

    def aug(vh):  # [H, DV] -> [C, NCH*(DV+1)] partition-major with ones col
        a = np.concatenate([vh, np.ones((H, 1), np.float32)], axis=1)
        return a.reshape(NCH, C, VW).transpose(1, 0, 2).reshape(C, NCH * VW)

    zeros_vp = np.zeros((C, NCH * VW), np.float32)
    zeros_ktp = np.zeros((D, H), np.float32)

    in_maps = []
    for core in range(NCORES):
        b_idx, half = divmod(core, 2)
        sl = slice(half * H, (half + 1) * H)
        b16 = np.empty((D, B16_COLS), np.float32)
        b16[:, OFF_WTB:OFF_WTB + D] = WT
        b16[:, OFF_MASK:OFF_MASK + C] = maskm
        b16[:, OFF_ID:OFF_ID + C] = ident
        b16[:, OFF_BIAS] = b_phi
        b16[:, OFF_QT:OFF_QT + H] = q[b_idx, sl].T
        b16[:, OFF_KT:OFF_KT + H] = k[b_idx, sl].T
        if half == 1:
            b16[:, OFF_KTP:OFF_KTP + H] = k[b_idx, 0:H].T
            b16[:, OFF_VP:OFF_VP + NCH * VW] = aug(v[b_idx, 0:H])
        else:
            b16[:, OFF_KTP:OFF_KTP + H] = zeros_ktp
            b16[:, OFF_VP:OFF_VP + NCH * VW] = zeros_vp
        b16[:, OFF_V:OFF_V + NCH * VW] = aug(v[b_idx, sl])
        in_maps.append({"bin": b16.astype(bf16), "btile": btile})
    return in_maps


def kernel(q, k, v, W_phi, b_phi):
    from concourse.bass_utils import run_bass_kernel_spmd

    q = np.asarray(q, np.float32)
    k = np.asarray(k, np.float32)
    v = np.asarray(v, np.float32)
    W_phi = np.asarray(W_phi, np.float32)
    b_phi = np.asarray(b_phi, np.float32)

    in_maps = _pack_inputs(q, k, v, W_phi, b_phi)
    nc = _get_nc()
    res = run_bass_kernel_spmd(nc, in_maps, list(range(NCORES)))

    out = np.empty((B, T, DV), np.float32)
    for core in range(NCORES):
        b_idx, half = divmod(core, 2)
        o = res.results[core]["out"]                      # [C, NCH*DV]
        o = o.reshape(C, NCH, DV).transpose(1, 0, 2).reshape(H, DV)
        out[b_idx, half * H:(half + 1) * H] = o
    return out
